# revision 29
# baseline (speedup 1.0000x reference)
"""Expert-choice MoE router kernel for Trainium2 (8 NeuronCores).

Problem (B=4, T=8192, D=512, E=8, H=2048, C=1024):
  scores = x @ Wg                         (B, T, E)
  w      = softmax(scores^T over T)       (B, E, T)
  top-C tokens per (b, e) by w            (expert choice)
  y_e    = gelu(x[sel] @ W1[e]) @ W2[e] * w[sel]
  out    = scatter_add(y_e) / max(scatter_add(w[sel]), 1e-8)

Sharding: batch x expert-group. Core c = 2b+p owns batch b = c//2 and
expert group p = c%2 (experts 4p..4p+3). All phases are per-batch local:
  - scores: each core computes its 4 experts' full-T scores from a
    host-pretransposed x[b] (no collective needed).
  - top-C: fixed-round threshold bisection on fp32 scores, vectorized
    over the 4 experts (128 partitions = 4 x 32).
  - compaction: gpsimd sparse_gather -> (16, C/16) idx/val in the wrapped
    order that dma_gather/dma_scatter_add consume natively.
  - FFN per expert: gpsimd dma_gather(transpose=True) pulls the selected
    tokens in bf16 directly into [d-partition, token-free] layout; two
    bf16 matmul chains with exact gelu; weighted output rows + gate value
    (513 f32) are accumulated into a per-core (T, 576) f32 buffer with
    gpsimd dma_scatter_add (cross-expert collisions accumulate in HBM).
  - combine: pairwise ReduceScatter(add) between cores 2b/2b+1 sums the
    two expert groups and splits T; each core normalizes its T/2 rows;
    host concatenates.
"""

import sys

sys.path.insert(0, "/opt/trn_rl_repo")

import numpy as np
import ml_dtypes

import concourse.bass as bass  # noqa: F401
import concourse.mybir as mybir
import concourse.tile as tile
from concourse import bacc
from concourse.bass import IndirectOffsetOnAxis
from concourse.bass_utils import run_bass_kernel_spmd
from concourse.tile import add_dep_helper

F32 = mybir.dt.float32
BF16 = mybir.dt.bfloat16
I16 = mybir.dt.int16
I32 = mybir.dt.int32
U32 = mybir.dt.uint32
AF = mybir.ActivationFunctionType
ALU = mybir.AluOpType

NCORES = 8

B, T, D, E, H, C = 4, 8192, 512, 8, 2048, 1024
EG = 4                  # experts per core (group)
TH = T // 2             # output rows per core
ROW = 520               # dense row width (f32)
PAY = D + 1             # meaningful columns (output + gate)
DC = D // 128           # 4
HC = H // 128           # 16
CF = C // 16            # 64  compacted columns
CS = C // 128           # 8   c-subtiles
TB16 = T // 16          # 512 w16 columns per expert
TPP = T // 32           # 256 tokens per partition in w128 layout
NROUNDS = 26
BIS_LO, BIS_HI = -8.0, 8.0


def build_nc(stage=3, nrounds=NROUNDS):
    nc = bacc.Bacc("TRN2", target_bir_lowering=False, debug=False,
                   num_devices=NCORES)

    # ---- I/O ----
    xt_d = nc.dram_tensor("xt", [D, T], F32, kind="ExternalInput")
    xbf_d = nc.dram_tensor("x_bf", [T, D], BF16, kind="ExternalInput")
    wg_d = nc.dram_tensor("wg", [D, EG], F32, kind="ExternalInput")
    w1_d = nc.dram_tensor("w1", [EG, D, H], BF16, kind="ExternalInput")
    w2_d = nc.dram_tensor("w2", [EG, H, D], BF16, kind="ExternalInput")
    iotap1_d = nc.dram_tensor("iotap1", [16, TB16], F32, kind="ExternalInput")
    e1_d = nc.dram_tensor("e1", [128, EG], F32, kind="ExternalInput")
    e2_d = nc.dram_tensor("e2", [EG, 128], F32, kind="ExternalInput")
    o416_d = nc.dram_tensor("o416", [EG, 16], F32, kind="ExternalInput")
    id4_d = nc.dram_tensor("id4", [EG, EG], I32, kind="ExternalInput")
    id16_d = nc.dram_tensor("id16", [16, 16], F32, kind="ExternalInput")
    idbf_d = nc.dram_tensor("idbf", [128, 128], BF16, kind="ExternalInput")
    o16_d = nc.dram_tensor("o16", [16, 1], F32, kind="ExternalInput")
    mk_d = nc.dram_tensor("mk", [CF, 128], F32, kind="ExternalInput")
    rsel_d = nc.dram_tensor("rsel", [CF, CS], F32, kind="ExternalInput")

    out_sh = nc.dram_tensor("out_sh", [TH, D], F32, kind="ExternalOutput")
    nf_out = nc.dram_tensor("nf_out", [EG, 2], U32, kind="ExternalOutput")
    dbg = {}
    if stage < 3:
        dbg["sc"] = nc.dram_tensor("dbg_sc", [EG, T], F32, kind="ExternalOutput")
        dbg["lo"] = nc.dram_tensor("dbg_lo", [EG, 1], F32, kind="ExternalOutput")
        dbg["idx"] = nc.dram_tensor("dbg_idx", [EG, 128, CS], I32,
                                    kind="ExternalOutput")
        dbg["val"] = nc.dram_tensor("dbg_val", [EG, 128, CS], F32,
                                    kind="ExternalOutput")
    if stage == 2:
        dbg["dense"] = nc.dram_tensor("dbg_dense", [T, ROW], F32,
                                      kind="ExternalOutput")

    # ---- internal DRAM ----
    scores_d = nc.dram_tensor("scores_d", [EG, T], F32)
    dense = nc.dram_tensor("dense", [T, ROW], F32)
    rs_out = nc.dram_tensor("rs_out", [TH, ROW], F32)

    with tile.TileContext(nc) as tc:
        with (
            tc.tile_pool(name="const", bufs=1) as cp,
            tc.tile_pool(name="sc", bufs=2) as scp,
            tc.tile_pool(name="bis", bufs=1) as bp,
            tc.tile_pool(name="wts", bufs=2) as wp,
            tc.tile_pool(name="ffn", bufs=2) as fp,
            tc.tile_pool(name="pk", bufs=2) as pkp,
            tc.tile_pool(name="norm", bufs=2) as np_,
            tc.tile_pool(name="pmm", bufs=2, space="PSUM") as pmm,
            tc.tile_pool(name="pps", bufs=3, space="PSUM") as pps,
        ):
            # ---------- constants (sync queue) ----------
            wg_sb = cp.tile([128, DC, EG], F32, tag="wg_sb")
            nc.sync.dma_start(wg_sb[:], wg_d.ap().rearrange("(c p) e -> p c e", p=128))
            iotap1 = cp.tile([16, TB16], F32, tag="iotap1")
            nc.sync.dma_start(iotap1[:], iotap1_d.ap())
            e1s = cp.tile([128, EG], F32, tag="e1s")
            nc.sync.dma_start(e1s[:], e1_d.ap())
            e2s = cp.tile([EG, 128], F32, tag="e2s")
            nc.sync.dma_start(e2s[:], e2_d.ap())
            o416 = cp.tile([EG, 16], F32, tag="o416")
            nc.sync.dma_start(o416[:], o416_d.ap())
            id4s = cp.tile([EG, EG], I32, tag="id4s")
            nc.sync.dma_start(id4s[:], id4_d.ap())
            id16s = cp.tile([16, 16], F32, tag="id16s")
            nc.sync.dma_start(id16s[:], id16_d.ap())
            idbfs = cp.tile([128, 128], BF16, tag="idbfs")
            nc.sync.dma_start(idbfs[:], idbf_d.ap())
            o16s = cp.tile([16, 1], F32, tag="o16s")
            nc.sync.dma_start(o16s[:], o16_d.ap())
            mks = cp.tile([CF, 128], F32, tag="mks")
            nc.sync.dma_start(mks[:], mk_d.ap())
            rsels = cp.tile([CF, CS], F32, tag="rsels")
            nc.sync.dma_start(rsels[:], rsel_d.ap())

            # ---------- phase 1: scores for my 4 experts, full T ----------
            last_xt_load = None
            for nt in range(T // 512):
                xt_t = scp.tile([128, DC, 512], F32, tag="xt")
                last_xt_load = nc.sync.dma_start(
                    xt_t[:],
                    xt_d.ap().rearrange("(c p) t -> p c t", p=128)[
                        :, :, nt * 512:(nt + 1) * 512],
                )
                ps_sc = pps.tile([EG, 512], F32, tag="sp")
                for dc in range(DC):
                    nc.tensor.matmul(ps_sc[:], lhsT=wg_sb[:, dc, :],
                                     rhs=xt_t[:, dc, :],
                                     start=(dc == 0), stop=(dc == DC - 1))
                sc_sb = scp.tile([EG, 512], F32, tag="scsb")
                nc.vector.tensor_copy(sc_sb[:], ps_sc[:])
                nc.sync.dma_start(scores_d.ap()[:, nt * 512:(nt + 1) * 512],
                                  sc_sb[:])

            # ---------- zero the dense accumulator (scalar queue) ----------
            # gated behind the xt stream so it doesn't steal HBM bandwidth
            # from the latency-critical score path
            zt = cp.tile([128, ROW], F32, tag="zt")
            zmem = nc.vector.memset(zt[:], 0.0)
            add_dep_helper(zmem.ins, last_xt_load.ins,
                           reason="delay zero behind xt")
            dense_z = dense.ap().rearrange("(j p) r -> j p r", p=128)
            for j in range(T // 128):
                nc.scalar.dma_start(dense_z[j], zt[:])

            # w128: (128, TPP); partition e*32 + l holds tokens [l*TPP, ...)
            w128 = cp.tile([128, TPP], F32, tag="w128")
            for e in range(EG):
                nc.sync.dma_start(
                    w128[e * 32:(e + 1) * 32, :],
                    scores_d.ap()[e].rearrange("(l m) -> l m", l=32))
            # w16: (16, TB16*EG); [s, e*TB16 + j] = scores[e, s*TB16 + j]
            w16 = cp.tile([16, EG * TB16], F32, tag="w16")
            for e in range(EG):
                nc.sync.dma_start(
                    w16[:, e * TB16:(e + 1) * TB16],
                    scores_d.ap()[e].rearrange("(s j) -> s j", s=16))
            if stage < 3:
                for e in range(EG):
                    dbt = np_.tile([16, TB16], F32, tag="dbt")
                    nc.sync.dma_start(
                        dbt[:], scores_d.ap()[e].rearrange("(s j) -> s j", s=16))
                    nc.sync.dma_start(
                        dbg["sc"].ap()[e].rearrange("(s j) -> s j", s=16), dbt[:])

            # ---------- phase 2: softmax pieces (exp + row sums) ----------
            exp16 = cp.tile([16, EG * TB16], F32, tag="exp16")
            parts16 = bp.tile([16, EG], F32, tag="parts16")
            for e in range(EG):
                sl = slice(e * TB16, (e + 1) * TB16)
                nc.scalar.activation(exp16[:, sl], w16[:, sl], AF.Exp,
                                     accum_out=parts16[:, e:e + 1])
            ps4 = pps.tile([EG, 1], F32, tag="sp")
            nc.tensor.matmul(ps4[:], lhsT=parts16[:], rhs=o16s[:],
                             start=True, stop=True)
            recip4 = bp.tile([EG, 1], F32, tag="recip4")
            nc.vector.reciprocal(recip4[:], ps4[:])
            diagr = bp.tile([EG, EG], F32, tag="diagr")
            nc.vector.memset(diagr[:], 0.0)
            nc.vector.copy_predicated(diagr[:], id4s[:],
                                      recip4[:, 0:1].to_broadcast([EG, EG]))
            psr16 = pps.tile([16, EG], F32, tag="sp")
            nc.tensor.matmul(psr16[:], lhsT=o416[:], rhs=diagr[:],
                             start=True, stop=True)
            recip16 = cp.tile([16, EG], F32, tag="recip16")
            nc.vector.tensor_copy(recip16[:], psr16[:])

            # ---------- phase 3: threshold bisection (top-C cut) -------
            lo4 = bp.tile([EG, 1], F32, tag="lo4")
            hi4 = bp.tile([EG, 1], F32, tag="hi4")
            nc.vector.memset(lo4[:], BIS_LO)
            nc.vector.memset(hi4[:], BIS_HI)
            mid4 = bp.tile([EG, 1], F32, tag="mid4")
            sel4 = bp.tile([EG, 1], I32, tag="sel4")
            seli4 = bp.tile([EG, 1], I32, tag="seli4")
            midbc = bp.tile([128, 1], F32, tag="midbc")
            cnt128 = bp.tile([128, 1], F32, tag="cnt128")
            msk = bp.tile([128, TPP], F32, tag="msk")
            for _ in range(nrounds):
                nc.vector.tensor_add(mid4[:], lo4[:], hi4[:])
                nc.vector.tensor_scalar_mul(mid4[:], mid4[:], 0.5)
                pmb = pps.tile([128, 1], F32, tag="sp")
                nc.tensor.matmul(pmb[:], lhsT=e2s[:], rhs=mid4[:],
                                 start=True, stop=True)
                nc.vector.tensor_copy(midbc[:], pmb[:])
                nc.vector.tensor_scalar(msk[:], w128[:], midbc[:, 0:1], None,
                                        op0=ALU.is_ge, op1=ALU.add,
                                        accum_out=cnt128[:, 0:1])
                pc4 = pps.tile([EG, 1], F32, tag="sp")
                nc.tensor.matmul(pc4[:], lhsT=e1s[:], rhs=cnt128[:],
                                 start=True, stop=True)
                nc.vector.tensor_scalar(sel4[:], pc4[:], float(C) - 0.5, None,
                                        op0=ALU.is_ge)
                nc.vector.tensor_scalar(seli4[:], pc4[:], float(C) - 0.5, None,
                                        op0=ALU.is_lt)
                nc.vector.copy_predicated(lo4[:], sel4[:], mid4[:])
                nc.vector.copy_predicated(hi4[:], seli4[:], mid4[:])

            # tau16[:, e] = lo4[e] replicated over 16 partitions
            diagt = bp.tile([EG, EG], F32, tag="diagt")
            nc.vector.memset(diagt[:], 0.0)
            nc.vector.copy_predicated(diagt[:], id4s[:],
                                      lo4[:, 0:1].to_broadcast([EG, EG]))
            pst16 = pps.tile([16, EG], F32, tag="sp")
            nc.tensor.matmul(pst16[:], lhsT=o416[:], rhs=diagt[:],
                             start=True, stop=True)
            tau16 = cp.tile([16, EG], F32, tag="tau16")
            nc.vector.tensor_copy(tau16[:], pst16[:])
            if stage < 3:
                nc.sync.dma_start(dbg["lo"].ap(), lo4[:])

            # ---------- phase 4: per-expert compact + FFN + scatter ----
            NT = 512
            for e in range(EG):
                # weight streams (sync queue, behind the xt chunks;
                # wts pool double-buffers)
                w1_sb = wp.tile([128, DC, H], BF16, tag="w1")
                nc.sync.dma_start(
                    w1_sb[:], w1_d.ap()[e].rearrange("(c p) h -> p c h", p=128))
                w2_sb = wp.tile([128, HC, D], BF16, tag="w2")
                nc.sync.dma_start(
                    w2_sb[:], w2_d.ap()[e].rearrange("(c p) d -> p c d", p=128))

                sl = slice(e * TB16, (e + 1) * TB16)
                mask16 = bp.tile([16, TB16], F32, tag="mask16")
                nc.vector.tensor_scalar(mask16[:], w16[:, sl], tau16[:, e:e + 1],
                                        None, op0=ALU.is_ge)
                candi = bp.tile([16, TB16], F32, tag="candi")
                nc.vector.tensor_tensor(candi[:], mask16[:], iotap1[:],
                                        op=ALU.mult)
                nc.vector.tensor_scalar_add(candi[:], candi[:], -1.0)
                candv = bp.tile([16, TB16], F32, tag="candv")
                nc.vector.tensor_tensor(candv[:], mask16[:], exp16[:, sl],
                                        op=ALU.mult)
                nc.vector.tensor_scalar_add(mask16[:], mask16[:], -1.0)
                nc.vector.tensor_tensor(candv[:], candv[:], mask16[:],
                                        op=ALU.add)

                ci = bp.tile([16, CF + 16], F32, tag=f"ci{e}")
                nfi = bp.tile([1, 1], U32, tag=f"nfi{e}")
                nc.gpsimd.sparse_gather(ci[:], candi[:], num_found=nfi[:])
                cv = bp.tile([16, CF + 16], F32, tag=f"cv{e}")
                nfv = bp.tile([1, 1], U32, tag=f"nfv{e}")
                nc.gpsimd.sparse_gather(cv[:], candv[:], num_found=nfv[:])
                nc.sync.dma_start(nf_out.ap()[e:e + 1, 0:1], nfi[:, :])
                nc.sync.dma_start(nf_out.ap()[e:e + 1, 1:2], nfv[:, :])

                # gate vals = exp * (1/rowsum)
                nc.vector.tensor_scalar(cv[:, :CF], cv[:, :CF],
                                        recip16[:, e:e + 1], None, op0=ALU.mult)

                # idx/val 16->128 relayout: transpose, replicate x8, mask, matmul
                outs = []
                for src in (ci, cv):
                    pt = pps.tile([CF, 16], F32, tag="sp")
                    nc.tensor.transpose(pt[:], src[:, :CF], id16s[:])
                    ct_ = bp.tile([CF, 16], F32, tag="cvt")
                    nc.vector.tensor_copy(ct_[:], pt[:])
                    cm = bp.tile([CF, 128], F32, tag="cmv")
                    nc.vector.tensor_tensor(
                        cm[:].rearrange("f (g s) -> f g s", g=8),
                        ct_[:, None, :].to_broadcast([CF, 8, 16]),
                        mks[:].rearrange("f (g s) -> f g s", g=8),
                        op=ALU.mult)
                    pr = pps.tile([128, CS], F32, tag="sp")
                    nc.tensor.matmul(pr[:], lhsT=cm[:], rhs=rsels[:],
                                     start=True, stop=True)
                    outs.append(pr)
                idx32 = cp.tile([128, CS], I32, name=f"idx32_{e}", tag=f"idx32_{e}")
                nc.vector.tensor_copy(idx32[:], outs[0][:])
                val128 = cp.tile([128, CS], F32, name=f"val128_{e}",
                                 tag=f"val128_{e}")
                nc.vector.tensor_copy(val128[:], outs[1][:])

                if stage < 3:
                    nc.sync.dma_start(dbg["idx"].ap()[e], idx32[:])
                    nc.sync.dma_start(dbg["val"].ap()[e], val128[:])
                if stage < 2:
                    continue

                # gather selected token rows (HW only supports (128,1)
                # offset APs), then PE-transpose to
                # selT[c0, dc, i] = x[idx_i, dc*128+c0]
                selTM = fp.tile([128, CS, D], BF16, tag="selTM", bufs=1)
                for cs in range(CS):
                    nc.gpsimd.indirect_dma_start(
                        out=selTM[:, cs, :],
                        out_offset=None,
                        in_=xbf_d.ap(),
                        in_offset=IndirectOffsetOnAxis(
                            ap=idx32[:, cs:cs + 1], axis=0))
                selT = fp.tile([128, DC, C], BF16, tag="selT")
                for cs in range(CS):
                    for dc in range(DC):
                        ptp = pps.tile([128, 128], BF16, tag="tp")
                        nc.tensor.transpose(
                            ptp[:], selTM[:, cs, dc * 128:(dc + 1) * 128],
                            idbfs[:])
                        nc.vector.tensor_copy(
                            selT[:, dc, cs * 128:(cs + 1) * 128], ptp[:])

                for ct in range(C // NT):
                    csl = slice(ct * NT, (ct + 1) * NT)
                    hT = fp.tile([128, HC, NT], BF16, tag="hT")
                    for ht in range(HC):
                        psh = pmm.tile([128, NT], F32, tag="mm")
                        for dc in range(DC):
                            nc.tensor.matmul(
                                psh[:],
                                lhsT=w1_sb[:, dc, ht * 128:(ht + 1) * 128],
                                rhs=selT[:, dc, csl],
                                start=(dc == 0), stop=(dc == DC - 1))
                        nc.scalar.activation(hT[:, ht, :], psh[:], AF.Gelu)
                    pk = pkp.tile([128, NT // 128, ROW], F32, tag="pk")
                    nc.vector.memset(pk[:, :, PAY:], 0.0)
                    for cl in range(NT // 128):
                        cs = ct * (NT // 128) + cl
                        pso = pmm.tile([128, D], F32, tag="mm")
                        for hc in range(HC):
                            nc.tensor.matmul(
                                pso[:],
                                lhsT=hT[:, hc, cl * 128:(cl + 1) * 128],
                                rhs=w2_sb[:, hc, :],
                                start=(hc == 0), stop=(hc == HC - 1))
                        nc.vector.tensor_scalar(
                            pk[:, cl, :D], pso[:],
                            val128[:, cs:cs + 1], None, op0=ALU.mult)
                        nc.vector.tensor_copy(pk[:, cl, D:D + 1],
                                              val128[:, cs:cs + 1])

                    # scatter-add full 520-wide rows into dense
                    # (accumulates in HBM; WAW-serialized across experts)
                    for cl in range(NT // 128):
                        cs = ct * (NT // 128) + cl
                        nc.gpsimd.indirect_dma_start(
                            out=dense.ap(),
                            out_offset=IndirectOffsetOnAxis(
                                ap=idx32[:, cs:cs + 1], axis=0),
                            in_=pk[:, cl, :],
                            in_offset=None,
                            bounds_check=T - 1,
                            oob_is_err=False,
                            compute_op=ALU.add)

            # ---------- phase 5: pair ReduceScatter + normalize ----------
            if stage == 2:
                dzi = dense.ap().rearrange("(j p) r -> j p r", p=128)
                dzo = dbg["dense"].ap().rearrange("(j p) r -> j p r", p=128)
                for j in range(T // 128):
                    dbd = np_.tile([128, ROW], F32, tag="dbd")
                    nc.sync.dma_start(dbd[:], dzi[j])
                    nc.sync.dma_start(dzo[j], dbd[:])
            if stage >= 3:
                nc.gpsimd.collective_compute(
                    "ReduceScatter", ALU.add,
                    replica_groups=[[0, 1], [2, 3], [4, 5], [6, 7]],
                    ins=[dense.ap()], outs=[rs_out.ap()],
                )
                NB = 4          # row-tiles per normalize step
                rs_v = rs_out.ap().rearrange("(j b p) r -> j b p r", b=NB, p=128)
                ou_v = out_sh.ap().rearrange("(j b p) d -> j b p d", b=NB, p=128)
                for j in range(TH // (128 * NB)):
                    ld = np_.tile([128, NB, PAY], F32, tag="ld")
                    nc.sync.dma_start(
                        ld[:], rs_v[j].rearrange("b p r -> p b r")[:, :, :PAY])
                    dn = np_.tile([128, NB], F32, tag="dn")
                    nc.vector.tensor_scalar(dn[:], ld[:, :, D:D + 1], 1e-8, None,
                                            op0=ALU.max)
                    rc = np_.tile([128, NB], F32, tag="rc")
                    nc.vector.reciprocal(rc[:], dn[:])
                    nc.vector.tensor_tensor(
                        ld[:, :, :D], ld[:, :, :D],
                        rc[:, :, None].to_broadcast([128, NB, D]),
                        op=ALU.mult)
                    nc.sync.dma_start(ou_v[j].rearrange("b p d -> p b d"),
                                      ld[:, :, :D])

    nc.compile()
    return nc


# ---------------------------------------------------------------------------
# host side
# ---------------------------------------------------------------------------

def host_consts():
    iotap1 = (np.arange(16)[:, None] * TB16
              + np.arange(TB16)[None, :] + 1).astype(np.float32)
    p = np.arange(128)
    e1 = (p[:, None] // 32 == np.arange(EG)[None, :]).astype(np.float32)
    e2 = np.ascontiguousarray(e1.T)
    o416 = np.ones((EG, 16), np.float32)
    id4 = np.eye(EG, dtype=np.int32)
    id16 = np.eye(16, dtype=np.float32)
    idbf = np.eye(128).astype(ml_dtypes.bfloat16)
    o16 = np.ones((16, 1), np.float32)
    f = np.arange(CF)
    g = np.arange(8)
    mk = np.zeros((CF, 128), np.float32)
    mk.reshape(CF, 8, 16)[:, :, :] = (f[:, None] % 8 == g[None, :]).astype(
        np.float32)[:, :, None]
    rsel = (f[:, None] // 8 == np.arange(CS)[None, :]).astype(np.float32)
    return dict(iotap1=iotap1, e1=e1, e2=e2, o416=o416, id4=id4, id16=id16,
                idbf=idbf, o16=o16, mk=mk, rsel=rsel)


def make_in_maps(inputs):
    x = np.asarray(inputs["x"], np.float32)
    Wg = np.asarray(inputs["Wg"], np.float32)
    W1 = np.asarray(inputs["W1"], np.float32)
    W2 = np.asarray(inputs["W2"], np.float32)
    consts = host_consts()
    xt = [np.ascontiguousarray(x[b].T) for b in range(B)]
    xbf = [np.ascontiguousarray(x[b]).astype(ml_dtypes.bfloat16)
           for b in range(B)]
    wg = [np.ascontiguousarray(Wg[:, p * EG:(p + 1) * EG]) for p in range(2)]
    w1 = [np.ascontiguousarray(W1[p * EG:(p + 1) * EG]).astype(
        ml_dtypes.bfloat16) for p in range(2)]
    w2 = [np.ascontiguousarray(W2[p * EG:(p + 1) * EG]).astype(
        ml_dtypes.bfloat16) for p in range(2)]
    in_maps = []
    for c in range(NCORES):
        b, p = divmod(c, 2)
        m = dict(consts)
        m["xt"] = xt[b]
        m["x_bf"] = xbf[b]
        m["wg"] = wg[p]
        m["w1"] = w1[p]
        m["w2"] = w2[p]
        in_maps.append(m)
    return in_maps


def assemble_out(results):
    nf = np.stack([np.asarray(results[i]["nf_out"]) for i in range(NCORES)])
    if not (nf == C).all():
        print(f"WARNING: sparse_gather num_found != {C}: {nf.tolist()}",
              file=sys.stderr)
    out = np.concatenate([results[i]["out_sh"] for i in range(NCORES)], 0)
    return np.ascontiguousarray(out.reshape(B, T, D), dtype=np.float32)


_NC_CACHE = {}


def get_nc():
    if "nc" not in _NC_CACHE:
        _NC_CACHE["nc"] = build_nc()
    return _NC_CACHE["nc"]


def kernel(**inputs):
    nc = get_nc()
    in_maps = make_in_maps(inputs)
    res = run_bass_kernel_spmd(nc, in_maps, core_ids=list(range(NCORES)),
                               **_NC_CACHE.get("run_kwargs", {}))
    _NC_CACHE["last_run"] = res
    return assemble_out(res.results)


# revision 46
# speedup vs baseline: 1.1795x; 1.1795x over previous
"""Expert-choice MoE router kernel for Trainium2 (8 NeuronCores).

Problem (B=4, T=8192, D=512, E=8, H=2048, C=1024):
  scores = x @ Wg                         (B, T, E)
  w      = softmax(scores^T over T)       (B, E, T)
  top-C tokens per (b, e) by w            (expert choice)
  y_e    = gelu(x[sel] @ W1[e]) @ W2[e] * w[sel]
  out    = scatter_add(y_e) / max(scatter_add(w[sel]), 1e-8)

Sharding: batch x expert-group. Core c = 2b+p owns batch b = c//2 and
expert group p = c%2 (experts 4p..4p+3). All phases are per-batch local:
  - scores: each core computes its 4 experts' full-T scores from a
    host-pretransposed x[b] (no collective needed).
  - top-C: fixed-round threshold bisection on fp32 scores, vectorized
    over the 4 experts (128 partitions = 4 x 32).
  - compaction: gpsimd sparse_gather -> (16, C/16) idx/val in the wrapped
    order that dma_gather/dma_scatter_add consume natively.
  - FFN per expert: gpsimd dma_gather(transpose=True) pulls the selected
    tokens in bf16 directly into [d-partition, token-free] layout; two
    bf16 matmul chains with exact gelu; weighted output rows + gate value
    (513 f32) are accumulated into a per-core (T, 576) f32 buffer with
    gpsimd dma_scatter_add (cross-expert collisions accumulate in HBM).
  - combine: pairwise ReduceScatter(add) between cores 2b/2b+1 sums the
    two expert groups and splits T; each core normalizes its T/2 rows;
    host concatenates.
"""

import sys

sys.path.insert(0, "/opt/trn_rl_repo")

import numpy as np
import ml_dtypes

import concourse.bass as bass  # noqa: F401
import concourse.mybir as mybir
import concourse.tile as tile
from concourse import bacc
from concourse.bass import IndirectOffsetOnAxis
from concourse.bass_utils import run_bass_kernel_spmd
from concourse.tile import add_dep_helper

F32 = mybir.dt.float32
BF16 = mybir.dt.bfloat16
I16 = mybir.dt.int16
I32 = mybir.dt.int32
U32 = mybir.dt.uint32
AF = mybir.ActivationFunctionType
ALU = mybir.AluOpType

NCORES = 8

B, T, D, E, H, C = 4, 8192, 512, 8, 2048, 1024
EG = 4                  # experts per core (group)
TH = T // 2             # output rows per core
ROW = 520               # dense row width (f32)
PAY = D + 1             # meaningful columns (output + gate)
DC = D // 128           # 4
HC = H // 128           # 16
CF = C // 16            # 64  compacted columns
CS = C // 128           # 8   c-subtiles
TB16 = T // 16          # 512 w16 columns per expert
TPP = T // 32           # 256 tokens per partition in w128 layout
NROUNDS = 6             # K-ary rounds: resolves 16/17^6 ~ 6.6e-8
KB = 16                 # thresholds tested per round
BIS_LO, BIS_HI = -8.0, 8.0


def build_nc(stage=3, nrounds=NROUNDS):
    nc = bacc.Bacc("TRN2", target_bir_lowering=False, debug=False,
                   num_devices=NCORES)

    # ---- I/O ----
    xt_d = nc.dram_tensor("xt", [D, T], F32, kind="ExternalInput")
    xbf_d = nc.dram_tensor("x_bf", [T, D], BF16, kind="ExternalInput")
    wg_d = nc.dram_tensor("wg", [D, EG], F32, kind="ExternalInput")
    w1_d = nc.dram_tensor("w1", [EG, D, H], BF16, kind="ExternalInput")
    w2_d = nc.dram_tensor("w2", [EG, H, D], BF16, kind="ExternalInput")
    iotap1_d = nc.dram_tensor("iotap1", [16, TB16], F32, kind="ExternalInput")
    e1_d = nc.dram_tensor("e1", [128, EG], F32, kind="ExternalInput")
    e2_d = nc.dram_tensor("e2", [EG, 128], F32, kind="ExternalInput")
    o416_d = nc.dram_tensor("o416", [EG, 16], F32, kind="ExternalInput")
    id4_d = nc.dram_tensor("id4", [EG, EG], I32, kind="ExternalInput")
    id16_d = nc.dram_tensor("id16", [16, 16], F32, kind="ExternalInput")
    idbf_d = nc.dram_tensor("idbf", [128, 128], BF16, kind="ExternalInput")
    o16_d = nc.dram_tensor("o16", [16, 1], F32, kind="ExternalInput")
    mk_d = nc.dram_tensor("mk", [CF, 128], F32, kind="ExternalInput")
    rsel_d = nc.dram_tensor("rsel", [CF, CS], F32, kind="ExternalInput")
    coefk_d = nc.dram_tensor("coefk", [EG, KB], F32, kind="ExternalInput")
    parT_d = nc.dram_tensor("parT", [128, 1], F32, kind="ExternalInput")
    zidx_d = nc.dram_tensor("zidx", [128, T // 512], I32, kind="ExternalInput")
    nidx_d = nc.dram_tensor("nidx", [128, TH // 256], I32, kind="ExternalInput")

    out_sh = nc.dram_tensor("out_sh", [TH, D], F32, kind="ExternalOutput")
    nf_out = nc.dram_tensor("nf_out", [EG, 2], U32, kind="ExternalOutput")
    dbg = {}
    if stage < 3:
        dbg["sc"] = nc.dram_tensor("dbg_sc", [EG, T], F32, kind="ExternalOutput")
        dbg["lo"] = nc.dram_tensor("dbg_lo", [EG, 1], F32, kind="ExternalOutput")
        dbg["idx"] = nc.dram_tensor("dbg_idx", [EG, 128, CS], I32,
                                    kind="ExternalOutput")
        dbg["val"] = nc.dram_tensor("dbg_val", [EG, 128, CS], F32,
                                    kind="ExternalOutput")
    if stage == 2:
        dbg["dense"] = nc.dram_tensor("dbg_dense", [2 * T, ROW], F32,
                                      kind="ExternalOutput")

    # ---- internal DRAM ----
    scores_d = nc.dram_tensor("scores_d", [EG, T], F32)
    # pair-shared accumulator: slab [parity*T, parity*T+T) is written by this
    # core; both cores of an HBM pair read both slabs during the combine
    dense = nc.dram_tensor("dense_sh", [2 * T, ROW], F32, addr_space="Shared")
    bar_in = nc.dram_tensor("bar_in", [1, 1], F32)
    bar_out = nc.dram_tensor("bar_out", [1, 1], F32)

    with tile.TileContext(nc) as tc:
        with (
            tc.tile_pool(name="const", bufs=1) as cp,
            tc.tile_pool(name="sc", bufs=2) as scp,
            tc.tile_pool(name="bis", bufs=1) as bp,
            tc.tile_pool(name="wts", bufs=2) as wp,
            tc.tile_pool(name="ffn", bufs=2) as fp,
            tc.tile_pool(name="pk", bufs=2) as pkp,
            tc.tile_pool(name="norm", bufs=2) as np_,
            tc.tile_pool(name="pmm", bufs=2, space="PSUM") as pmm,
            tc.tile_pool(name="pps", bufs=3, space="PSUM") as pps,
        ):
            # ---------- constants (sync queue) ----------
            wg_sb = cp.tile([128, DC, EG], F32, tag="wg_sb")
            nc.sync.dma_start(wg_sb[:], wg_d.ap().rearrange("(c p) e -> p c e", p=128))
            iotap1 = cp.tile([16, TB16], F32, tag="iotap1")
            nc.sync.dma_start(iotap1[:], iotap1_d.ap())
            e1s = cp.tile([128, EG], F32, tag="e1s")
            nc.sync.dma_start(e1s[:], e1_d.ap())
            e2s = cp.tile([EG, 128], F32, tag="e2s")
            nc.sync.dma_start(e2s[:], e2_d.ap())
            o416 = cp.tile([EG, 16], F32, tag="o416")
            nc.sync.dma_start(o416[:], o416_d.ap())
            id4s = cp.tile([EG, EG], I32, tag="id4s")
            nc.sync.dma_start(id4s[:], id4_d.ap())
            id16s = cp.tile([16, 16], F32, tag="id16s")
            nc.sync.dma_start(id16s[:], id16_d.ap())
            idbfs = cp.tile([128, 128], BF16, tag="idbfs")
            nc.sync.dma_start(idbfs[:], idbf_d.ap())
            o16s = cp.tile([16, 1], F32, tag="o16s")
            nc.sync.dma_start(o16s[:], o16_d.ap())
            mks = cp.tile([CF, 128], F32, tag="mks")
            nc.sync.dma_start(mks[:], mk_d.ap())
            rsels = cp.tile([CF, CS], F32, tag="rsels")
            nc.sync.dma_start(rsels[:], rsel_d.ap())
            coefks = cp.tile([EG, KB], F32, tag="coefks")
            nc.sync.dma_start(coefks[:], coefk_d.ap())
            parTs = cp.tile([128, 1], F32, tag="parTs")
            nc.sync.dma_start(parTs[:], parT_d.ap())
            zidxs = cp.tile([128, T // 512], I32, tag="zidxs")
            nc.sync.dma_start(zidxs[:], zidx_d.ap())
            nidxs = cp.tile([128, TH // 256], I32, tag="nidxs")
            nc.sync.dma_start(nidxs[:], nidx_d.ap())
            # (2T/4, 2080) super-row view of the shared accumulator
            dense4 = dense.ap().rearrange("(q f) r -> q (f r)", f=4)

            # ---------- phase 1: scores for my 4 experts, full T ----------
            last_xt_load = None
            for nt in range(T // 512):
                xt_t = scp.tile([128, DC, 512], F32, tag="xt")
                last_xt_load = nc.sync.dma_start(
                    xt_t[:],
                    xt_d.ap().rearrange("(c p) t -> p c t", p=128)[
                        :, :, nt * 512:(nt + 1) * 512],
                )
                ps_sc = pps.tile([EG, 512], F32, tag="sp")
                for dc in range(DC):
                    nc.tensor.matmul(ps_sc[:], lhsT=wg_sb[:, dc, :],
                                     rhs=xt_t[:, dc, :],
                                     start=(dc == 0), stop=(dc == DC - 1))
                sc_sb = scp.tile([EG, 512], F32, tag="scsb")
                nc.vector.tensor_copy(sc_sb[:], ps_sc[:])
                nc.sync.dma_start(scores_d.ap()[:, nt * 512:(nt + 1) * 512],
                                  sc_sb[:])

            # ---------- zero my slab of the shared accumulator ----------
            # indirect scatters (host indices carry the parity offset), gated
            # behind the xt stream so they don't steal HBM bandwidth from the
            # latency-critical score path
            zt4 = cp.tile([128, 4 * ROW], F32, tag="zt4")
            zmem = nc.vector.memset(zt4[:], 0.0)
            add_dep_helper(zmem.ins, last_xt_load.ins,
                           reason="delay zero behind xt")
            for j in range(T // 512):
                nc.gpsimd.indirect_dma_start(
                    out=dense4,
                    out_offset=IndirectOffsetOnAxis(
                        ap=zidxs[:, j:j + 1], axis=0),
                    in_=zt4[:],
                    in_offset=None,
                    bounds_check=2 * T // 4 - 1,
                    oob_is_err=False)

            # w128: (128, TPP); partition e*32 + l holds tokens [l*TPP, ...)
            w128 = cp.tile([128, TPP], F32, tag="w128")
            for e in range(EG):
                nc.sync.dma_start(
                    w128[e * 32:(e + 1) * 32, :],
                    scores_d.ap()[e].rearrange("(l m) -> l m", l=32))
            # w16: (16, TB16*EG); [s, e*TB16 + j] = scores[e, s*TB16 + j]
            w16 = cp.tile([16, EG * TB16], F32, tag="w16")
            for e in range(EG):
                nc.sync.dma_start(
                    w16[:, e * TB16:(e + 1) * TB16],
                    scores_d.ap()[e].rearrange("(s j) -> s j", s=16))
            if stage < 3:
                for e in range(EG):
                    dbt = np_.tile([16, TB16], F32, tag="dbt")
                    nc.sync.dma_start(
                        dbt[:], scores_d.ap()[e].rearrange("(s j) -> s j", s=16))
                    nc.sync.dma_start(
                        dbg["sc"].ap()[e].rearrange("(s j) -> s j", s=16), dbt[:])

            # ---------- phase 2: softmax pieces (exp + row sums) ----------
            exp16 = cp.tile([16, EG * TB16], F32, tag="exp16")
            parts16 = bp.tile([16, EG], F32, tag="parts16")
            for e in range(EG):
                sl = slice(e * TB16, (e + 1) * TB16)
                nc.scalar.activation(exp16[:, sl], w16[:, sl], AF.Exp,
                                     accum_out=parts16[:, e:e + 1])
            ps4 = pps.tile([EG, 1], F32, tag="sp")
            nc.tensor.matmul(ps4[:], lhsT=parts16[:], rhs=o16s[:],
                             start=True, stop=True)
            recip4 = bp.tile([EG, 1], F32, tag="recip4")
            nc.vector.reciprocal(recip4[:], ps4[:])
            diagr = bp.tile([EG, EG], F32, tag="diagr")
            nc.vector.memset(diagr[:], 0.0)
            nc.vector.copy_predicated(diagr[:], id4s[:],
                                      recip4[:, 0:1].to_broadcast([EG, EG]))
            psr16 = pps.tile([16, EG], F32, tag="sp")
            nc.tensor.matmul(psr16[:], lhsT=o416[:], rhs=diagr[:],
                             start=True, stop=True)
            recip16 = cp.tile([16, EG], F32, tag="recip16")
            nc.vector.tensor_copy(recip16[:], psr16[:])

            # ---------- phase 3: K-ary threshold search (top-C cut) -----
            # each round tests KB thresholds theta_k = lo + (k+1)*step
            # simultaneously; counts are monotone in k, so the bracket is
            # recovered from m = #{k : count_k >= C} without a reduce-max.
            lo4 = bp.tile([EG, 1], F32, tag="lo4")
            hi4 = bp.tile([EG, 1], F32, tag="hi4")
            nc.vector.memset(lo4[:], BIS_LO)
            nc.vector.memset(hi4[:], BIS_HI)
            dif4 = bp.tile([EG, 1], F32, tag="dif4")
            step4 = bp.tile([EG, 1], F32, tag="step4")
            theta = bp.tile([EG, KB], F32, tag="theta")
            th128 = bp.tile([128, KB], F32, tag="th128")
            cnt128 = bp.tile([128, KB], F32, tag="cnt128")
            msk3 = bp.tile([128, KB // 4, TPP], BF16, tag="msk3")
            mdum = bp.tile([EG, KB], F32, tag="mdum")
            m4 = bp.tile([EG, 1], F32, tag="m4")
            tmp4 = bp.tile([EG, 1], F32, tag="tmp4")
            for _ in range(nrounds):
                nc.vector.tensor_tensor(dif4[:], hi4[:], lo4[:],
                                        op=ALU.subtract)
                nc.vector.tensor_scalar_mul(step4[:], dif4[:], 1.0 / (KB + 1))
                nc.vector.tensor_tensor(
                    theta[:], step4[:, 0:1].to_broadcast([EG, KB]), coefks[:],
                    op=ALU.mult)
                nc.vector.tensor_tensor(
                    theta[:], theta[:], lo4[:, 0:1].to_broadcast([EG, KB]),
                    op=ALU.add)
                pmb = pps.tile([128, KB], F32, tag="sp")
                nc.tensor.matmul(pmb[:], lhsT=e2s[:], rhs=theta[:],
                                 start=True, stop=True)
                nc.vector.tensor_copy(th128[:], pmb[:])
                for h in range(4):
                    hsl = slice(h * (KB // 4), (h + 1) * (KB // 4))
                    nc.vector.tensor_tensor(
                        msk3[:],
                        w128[:, None, :].to_broadcast([128, KB // 4, TPP]),
                        th128[:, hsl, None].to_broadcast([128, KB // 4, TPP]),
                        op=ALU.is_ge)
                    nc.vector.tensor_reduce(
                        cnt128[:, hsl], msk3[:],
                        axis=mybir.AxisListType.X, op=ALU.add)
                pc = pps.tile([EG, KB], F32, tag="sp")
                nc.tensor.matmul(pc[:], lhsT=e1s[:], rhs=cnt128[:],
                                 start=True, stop=True)
                nc.vector.tensor_scalar(mdum[:], pc[:], float(C) - 0.5, None,
                                        op0=ALU.is_ge, op1=ALU.add,
                                        accum_out=m4[:])
                nc.vector.tensor_tensor(tmp4[:], m4[:], step4[:], op=ALU.mult)
                nc.vector.tensor_add(lo4[:], lo4[:], tmp4[:])
                nc.vector.tensor_add(hi4[:], lo4[:], step4[:])

            # tau16[:, e] = lo4[e] replicated over 16 partitions
            diagt = bp.tile([EG, EG], F32, tag="diagt")
            nc.vector.memset(diagt[:], 0.0)
            nc.vector.copy_predicated(diagt[:], id4s[:],
                                      lo4[:, 0:1].to_broadcast([EG, EG]))
            pst16 = pps.tile([16, EG], F32, tag="sp")
            nc.tensor.matmul(pst16[:], lhsT=o416[:], rhs=diagt[:],
                             start=True, stop=True)
            tau16 = cp.tile([16, EG], F32, tag="tau16")
            nc.vector.tensor_copy(tau16[:], pst16[:])
            if stage < 3:
                nc.sync.dma_start(dbg["lo"].ap(), lo4[:])

            # ---------- phase 4: per-expert compact + FFN + scatter ----
            NT = 512
            scatter_insts = []
            for e in range(EG):
                # weight streams (sync queue, behind the xt chunks;
                # wts pool double-buffers)
                w1_sb = wp.tile([128, DC, H], BF16, tag="w1")
                nc.sync.dma_start(
                    w1_sb[:], w1_d.ap()[e].rearrange("(c p) h -> p c h", p=128))
                w2_sb = wp.tile([128, HC, D], BF16, tag="w2")
                nc.sync.dma_start(
                    w2_sb[:], w2_d.ap()[e].rearrange("(c p) d -> p c d", p=128))

                sl = slice(e * TB16, (e + 1) * TB16)
                mask16 = bp.tile([16, TB16], F32, tag="mask16")
                nc.vector.tensor_scalar(mask16[:], w16[:, sl], tau16[:, e:e + 1],
                                        None, op0=ALU.is_ge)
                candi = bp.tile([16, TB16], F32, tag="candi")
                nc.vector.tensor_tensor(candi[:], mask16[:], iotap1[:],
                                        op=ALU.mult)
                nc.vector.tensor_scalar_add(candi[:], candi[:], -1.0)
                candv = bp.tile([16, TB16], F32, tag="candv")
                nc.vector.tensor_tensor(candv[:], mask16[:], exp16[:, sl],
                                        op=ALU.mult)
                nc.vector.tensor_scalar_add(mask16[:], mask16[:], -1.0)
                nc.vector.tensor_tensor(candv[:], candv[:], mask16[:],
                                        op=ALU.add)

                ci = bp.tile([16, CF + 16], F32, tag=f"ci{e}")
                nfi = bp.tile([1, 1], U32, tag=f"nfi{e}")
                nc.gpsimd.sparse_gather(ci[:], candi[:], num_found=nfi[:])
                cv = bp.tile([16, CF + 16], F32, tag=f"cv{e}")
                nfv = bp.tile([1, 1], U32, tag=f"nfv{e}")
                nc.gpsimd.sparse_gather(cv[:], candv[:], num_found=nfv[:])
                nc.sync.dma_start(nf_out.ap()[e:e + 1, 0:1], nfi[:, :])
                nc.sync.dma_start(nf_out.ap()[e:e + 1, 1:2], nfv[:, :])

                # gate vals = exp * (1/rowsum)
                nc.vector.tensor_scalar(cv[:, :CF], cv[:, :CF],
                                        recip16[:, e:e + 1], None, op0=ALU.mult)

                # idx/val 16->128 relayout: transpose, replicate x8, mask, matmul
                outs = []
                for src in (ci, cv):
                    pt = pps.tile([CF, 16], F32, tag="sp")
                    nc.tensor.transpose(pt[:], src[:, :CF], id16s[:])
                    ct_ = bp.tile([CF, 16], F32, tag="cvt")
                    nc.vector.tensor_copy(ct_[:], pt[:])
                    cm = bp.tile([CF, 128], F32, tag="cmv")
                    nc.vector.tensor_tensor(
                        cm[:].rearrange("f (g s) -> f g s", g=8),
                        ct_[:, None, :].to_broadcast([CF, 8, 16]),
                        mks[:].rearrange("f (g s) -> f g s", g=8),
                        op=ALU.mult)
                    pr = pps.tile([128, CS], F32, tag="sp")
                    nc.tensor.matmul(pr[:], lhsT=cm[:], rhs=rsels[:],
                                     start=True, stop=True)
                    outs.append(pr)
                idx32 = cp.tile([128, CS], I32, name=f"idx32_{e}", tag=f"idx32_{e}")
                nc.vector.tensor_copy(idx32[:], outs[0][:])
                # scatter variant carries the slab (parity) row offset
                idx32s = cp.tile([128, CS], I32, name=f"idx32s_{e}",
                                 tag=f"idx32s_{e}")
                nc.vector.tensor_scalar(idx32s[:], outs[0][:], parTs[:, 0:1],
                                        None, op0=ALU.add)
                val128 = cp.tile([128, CS], F32, name=f"val128_{e}",
                                 tag=f"val128_{e}")
                nc.vector.tensor_copy(val128[:], outs[1][:])

                if stage < 3:
                    nc.sync.dma_start(dbg["idx"].ap()[e], idx32[:])
                    nc.sync.dma_start(dbg["val"].ap()[e], val128[:])
                if stage < 2:
                    continue

                # gather selected token rows (HW only supports (128,1)
                # offset APs), then PE-transpose to
                # selT[c0, dc, i] = x[idx_i, dc*128+c0]
                selTM = fp.tile([128, CS, D], BF16, tag="selTM", bufs=1)
                for cs in range(CS):
                    nc.gpsimd.indirect_dma_start(
                        out=selTM[:, cs, :],
                        out_offset=None,
                        in_=xbf_d.ap(),
                        in_offset=IndirectOffsetOnAxis(
                            ap=idx32[:, cs:cs + 1], axis=0))
                selT = fp.tile([128, DC, C], BF16, tag="selT", bufs=1)
                for cs in range(CS):
                    for dc in range(DC):
                        ptp = pps.tile([128, 128], BF16, tag="tp")
                        nc.tensor.transpose(
                            ptp[:], selTM[:, cs, dc * 128:(dc + 1) * 128],
                            idbfs[:])
                        nc.vector.tensor_copy(
                            selT[:, dc, cs * 128:(cs + 1) * 128], ptp[:])

                for ct in range(C // NT):
                    csl = slice(ct * NT, (ct + 1) * NT)
                    hT = fp.tile([128, HC, NT], BF16, tag="hT", bufs=1)
                    for ht in range(HC):
                        psh = pmm.tile([128, NT], F32, tag="mm")
                        for dc in range(DC):
                            nc.tensor.matmul(
                                psh[:],
                                lhsT=w1_sb[:, dc, ht * 128:(ht + 1) * 128],
                                rhs=selT[:, dc, csl],
                                start=(dc == 0), stop=(dc == DC - 1))
                        nc.scalar.activation(hT[:, ht, :], psh[:], AF.Gelu)
                    pk = pkp.tile([128, NT // 128, ROW], F32, tag="pk")
                    nc.vector.memset(pk[:, :, PAY:], 0.0)
                    for cl in range(NT // 128):
                        cs = ct * (NT // 128) + cl
                        pso = pmm.tile([128, D], F32, tag="mm")
                        for hc in range(HC):
                            nc.tensor.matmul(
                                pso[:],
                                lhsT=hT[:, hc, cl * 128:(cl + 1) * 128],
                                rhs=w2_sb[:, hc, :],
                                start=(hc == 0), stop=(hc == HC - 1))
                        nc.vector.tensor_scalar(
                            pk[:, cl, :D], pso[:],
                            val128[:, cs:cs + 1], None, op0=ALU.mult)
                        nc.vector.tensor_copy(pk[:, cl, D:D + 1],
                                              val128[:, cs:cs + 1])

                    # scatter-add full 520-wide rows into my shared slab
                    # (accumulates in HBM; WAW-serialized across experts)
                    for cl in range(NT // 128):
                        cs = ct * (NT // 128) + cl
                        sc_inst = nc.gpsimd.indirect_dma_start(
                            out=dense.ap(),
                            out_offset=IndirectOffsetOnAxis(
                                ap=idx32s[:, cs:cs + 1], axis=0),
                            in_=pk[:, cl, :],
                            in_offset=None,
                            bounds_check=2 * T - 1,
                            oob_is_err=False,
                            compute_op=ALU.add)
                        scatter_insts.append(sc_inst)

            # ---------- phase 5: pair barrier + shared-HBM combine -------
            if stage == 2:
                dzi = dense.ap().rearrange("(j p) r -> j p r", p=128)
                dzo = dbg["dense"].ap().rearrange("(j p) r -> j p r", p=128)
                for j in range(2 * T // 128):
                    dbd = np_.tile([128, ROW], F32, tag="dbd")
                    nc.sync.dma_start(dbd[:], dzi[j])
                    nc.sync.dma_start(dzo[j], dbd[:])
            if stage >= 3:
                # tiny pair AllReduce as a barrier: completion implies the
                # peer finished all its slab scatters
                bt = np_.tile([1, 1], F32, tag="bt", bufs=1)
                nc.vector.memset(bt[:], 1.0)
                bw = nc.sync.dma_start(bar_in.ap(), bt[:])
                for si in scatter_insts:
                    add_dep_helper(bw.ins, si.ins,
                                   reason="barrier waits for slab scatters")
                cc = nc.gpsimd.collective_compute(
                    "AllReduce", ALU.add,
                    replica_groups=[[0, 1], [2, 3], [4, 5], [6, 7]],
                    ins=[bar_in.ap()], outs=[bar_out.ap()])
                # gather both slabs' rows for my token half (4 consecutive
                # 520-f32 rows per descriptor), sum, normalize, store
                ou_v = out_sh.ap().rearrange("(j p f) d -> j p f d",
                                             p=128, f=4)
                for j in range(TH // 512):
                    ld = np_.tile([128, 2, 4 * ROW], F32, tag="ld")
                    for h in range(2):
                        g = nc.gpsimd.indirect_dma_start(
                            out=ld[:, h, :],
                            out_offset=None,
                            in_=dense4,
                            in_offset=IndirectOffsetOnAxis(
                                ap=nidxs[:, h * (TH // 512) + j:
                                         h * (TH // 512) + j + 1], axis=0))
                        add_dep_helper(g.ins, cc.ins,
                                       reason="read slabs after pair barrier")
                    nc.vector.tensor_tensor(ld[:, 0, :], ld[:, 0, :],
                                            ld[:, 1, :], op=ALU.add)
                    ldv = ld[:].rearrange("p h (f r) -> p h f r", f=4)
                    dn = np_.tile([128, 4], F32, tag="dn")
                    nc.vector.tensor_scalar(dn[:], ldv[:, 0, :, D:D + 1],
                                            1e-8, None, op0=ALU.max)
                    rc = np_.tile([128, 4], F32, tag="rc")
                    nc.vector.reciprocal(rc[:], dn[:])
                    nc.vector.tensor_tensor(
                        ldv[:, 0, :, :D], ldv[:, 0, :, :D],
                        rc[:, :, None].to_broadcast([128, 4, D]),
                        op=ALU.mult)
                    nc.sync.dma_start(ou_v[j], ldv[:, 0, :, :D])

    nc.compile()
    return nc


# ---------------------------------------------------------------------------
# host side
# ---------------------------------------------------------------------------

def host_consts():
    iotap1 = (np.arange(16)[:, None] * TB16
              + np.arange(TB16)[None, :] + 1).astype(np.float32)
    p = np.arange(128)
    e1 = (p[:, None] // 32 == np.arange(EG)[None, :]).astype(np.float32)
    e2 = np.ascontiguousarray(e1.T)
    o416 = np.ones((EG, 16), np.float32)
    id4 = np.eye(EG, dtype=np.int32)
    id16 = np.eye(16, dtype=np.float32)
    idbf = np.eye(128).astype(ml_dtypes.bfloat16)
    o16 = np.ones((16, 1), np.float32)
    f = np.arange(CF)
    g = np.arange(8)
    mk = np.zeros((CF, 128), np.float32)
    mk.reshape(CF, 8, 16)[:, :, :] = (f[:, None] % 8 == g[None, :]).astype(
        np.float32)[:, :, None]
    rsel = (f[:, None] // 8 == np.arange(CS)[None, :]).astype(np.float32)
    coefk = np.tile(np.arange(1, KB + 1, dtype=np.float32), (EG, 1))
    return dict(iotap1=iotap1, e1=e1, e2=e2, o416=o416, id4=id4, id16=id16,
                idbf=idbf, o16=o16, mk=mk, rsel=rsel, coefk=coefk)


def parity_inputs(p):
    pp = np.arange(128, dtype=np.int32)
    parT = np.full((128, 1), p * T, np.float32)
    zidx = (p * (T // 4) + np.arange(T // 512, dtype=np.int32)[None, :] * 128
            + pp[:, None]).astype(np.int32)
    nidx = np.zeros((128, TH // 256), np.int32)
    nj = TH // 512
    for h in range(2):
        for j in range(nj):
            nidx[:, h * nj + j] = h * (T // 4) + p * (TH // 4) + j * 128 + pp
    return dict(parT=parT, zidx=zidx, nidx=nidx)


def make_in_maps(inputs):
    x = np.asarray(inputs["x"], np.float32)
    Wg = np.asarray(inputs["Wg"], np.float32)
    W1 = np.asarray(inputs["W1"], np.float32)
    W2 = np.asarray(inputs["W2"], np.float32)
    consts = host_consts()
    xt = [np.ascontiguousarray(x[b].T) for b in range(B)]
    xbf = [np.ascontiguousarray(x[b]).astype(ml_dtypes.bfloat16)
           for b in range(B)]
    wg = [np.ascontiguousarray(Wg[:, p * EG:(p + 1) * EG]) for p in range(2)]
    w1 = [np.ascontiguousarray(W1[p * EG:(p + 1) * EG]).astype(
        ml_dtypes.bfloat16) for p in range(2)]
    w2 = [np.ascontiguousarray(W2[p * EG:(p + 1) * EG]).astype(
        ml_dtypes.bfloat16) for p in range(2)]
    pins = [parity_inputs(0), parity_inputs(1)]
    in_maps = []
    for c in range(NCORES):
        b, p = divmod(c, 2)
        m = dict(consts)
        m.update(pins[p])
        m["xt"] = xt[b]
        m["x_bf"] = xbf[b]
        m["wg"] = wg[p]
        m["w1"] = w1[p]
        m["w2"] = w2[p]
        in_maps.append(m)
    return in_maps


def assemble_out(results):
    nf = np.stack([np.asarray(results[i]["nf_out"]) for i in range(NCORES)])
    if not (nf == C).all():
        print(f"WARNING: sparse_gather num_found != {C}: {nf.tolist()}",
              file=sys.stderr)
    out = np.concatenate([results[i]["out_sh"] for i in range(NCORES)], 0)
    return np.ascontiguousarray(out.reshape(B, T, D), dtype=np.float32)


_NC_CACHE = {}


def get_nc():
    if "nc" not in _NC_CACHE:
        _NC_CACHE["nc"] = build_nc()
    return _NC_CACHE["nc"]


def kernel(**inputs):
    nc = get_nc()
    in_maps = make_in_maps(inputs)
    res = run_bass_kernel_spmd(nc, in_maps, core_ids=list(range(NCORES)),
                               **_NC_CACHE.get("run_kwargs", {}))
    _NC_CACHE["last_run"] = res
    return assemble_out(res.results)
